# revision 14
# baseline (speedup 1.0000x reference)
"""MoE layer (8 experts, top-2) for 8 Trainium2 NeuronCores — v2.

Expert-parallel (host router + gather as in v1), bf16 compute with
weight-stationary inner loops: for each weight tile, ALL token blocks are
streamed through the PE before the next LDWEIGHTS, cutting LDWEIGHTS count
~5x and keeping the tensor engine at the pure N-cycle streaming rate.

Stage B (x @ W1 -> gelu -> h) runs over all token blocks per fc before
advancing; h for the whole token set stays resident in SBUF (bf16).
Stage C (h @ W2 -> +b2 -> *scl -> yT) follows the same pattern.
"""

import os

import numpy as np

HIDDEN = 1024
FF = 2 * HIDDEN
NUM_EXPERTS = 8
TOP_K = 2
NCORES = 8

LAST_EXEC_NS = None
LAST_RESULTS = None

_PROGRAM_CACHE = {}


def _round_up(v, m):
    return (v + m - 1) // m * m


def _build_program(C, blk, repeat=1):
    """Bass/Tile program: one expert MLP over C tokens (SPMD on 8 cores).

    Layouts (per core):
      xT  [HIDDEN, C] bf16   - gathered tokens, transposed
      w1p [128, 8, HC, 2*128] bf16 (packed), b1 [FF] f32
      w2p [128, 8, 2, HIDDEN] bf16 (packed), b2 [HIDDEN] f32
      scl [C] f32            - per-token combine weight
      yT  [HIDDEN, C] f32    - output, transposed

    B: h[f, t] = gelu(sum_h w1[h, f] * xT[h, t] + b1[f])
    C: yT[o, t] = (sum_f w2[f, o] * h[f, t] + b2[o]) * scl[t]
    """
    import concourse.bass as bass  # noqa: F401
    import concourse.mybir as mybir
    import concourse.tile as tile
    from concourse import bacc

    HC = HIDDEN // 128  # 8 h-chunks
    FC = FF // 128  # 16 f-chunks
    f32 = mybir.dt.float32
    bf16 = mybir.dt.bfloat16

    nc = bacc.Bacc("TRN2", target_bir_lowering=False, debug=False,
                   num_devices=NCORES)
    W1G, W2G = 8, 8
    FPER = FC // W1G  # 2 f-chunks per w1 group
    CPER = FC // W2G  # 2 fc-chunks per w2 group
    xT = nc.dram_tensor("xT", [HIDDEN, C], bf16, kind="ExternalInput")
    w1 = nc.dram_tensor(
        "w1p", [128, W1G, HC, FPER * 128], bf16, kind="ExternalInput")
    b1 = nc.dram_tensor("b1", [FF], f32, kind="ExternalInput")
    w2 = nc.dram_tensor(
        "w2p", [128, W2G, CPER, HIDDEN], bf16, kind="ExternalInput")
    b2 = nc.dram_tensor("b2", [HIDDEN], f32, kind="ExternalInput")
    scl = nc.dram_tensor("scl", [C], f32, kind="ExternalInput")
    yT = nc.dram_tensor("yT", [HIDDEN, C], f32, kind="ExternalOutput")

    # Token blocks of `blk`, last one ragged (any remainder width is fine
    # in bf16; it is a small fraction of the stream).
    blocks = []
    t0 = 0
    while t0 < C:
        b = min(blk, C - t0)
        blocks.append((t0, b))
        t0 += b
    NB = len(blocks)

    Gelu = mybir.ActivationFunctionType.Gelu
    Ident = mybir.ActivationFunctionType.Identity

    with tile.TileContext(nc) as tc:
        with (
            tc.tile_pool(name="wts", bufs=1) as wts,
            tc.tile_pool(name="xin", bufs=1) as xin,
            tc.tile_pool(name="hmid", bufs=1) as hmid,
            tc.tile_pool(name="outs", bufs=4) as outs,
            tc.tile_pool(name="ps", bufs=8, space="PSUM") as ps,
        ):
            # --- PE clock warm-up: the HAM clock gate needs ~3.4us of
            # activity to switch 1.2->2.4 GHz; spend the DMA-latency
            # startup window on dummy matmuls so real MMs start warm.
            warm = wts.tile([128, 64], bf16, tag="warm")
            nc.vector.memset(warm[:], 0.0)
            for i in range(48):
                pw = ps.tile([128, 64], f32, tag="ps", name=f"warm{i}",
                             padded_shape=[128, blk])
                nc.tensor.matmul(pw[:64, :], warm[:], warm[:],
                                 start=True, stop=True)

            def emit_x(i, t0, bs, ring):
                xc = xin.tile([128, HC, bs], bf16, tag=f"xb{i}",
                              name=f"xb{i}")
                # two half DMAs: first 4 h-chunks land ~1.5us sooner,
                # so stage B's first matmuls start earlier
                for half in range(2):
                    ring.dma_start(
                        out=xc[:, half * 4:(half + 1) * 4, :],
                        in_=xT.ap().rearrange(
                            "(c p) t -> p c t", p=128)[
                            :, half * 4:(half + 1) * 4, t0:t0 + bs])
                return xc

            # --- biases first on sync ring (tiny), then weight stream.
            # Rep-0 x blocks 2+ ride the sync ring between w1 group 0 and
            # the rest (group g isn't consumed until ~17us*g, so x for
            # stage B's first pass takes priority); x blocks 0-1 + scl go
            # on the otherwise-idle scalar ring in parallel.
            b1_sb = wts.tile([128, FC], f32)
            nc.sync.dma_start(
                out=b1_sb[:], in_=b1.ap().rearrange("(c p) -> p c", p=128))
            b2_sb = wts.tile([128, HC], f32)
            nc.sync.dma_start(
                out=b2_sb[:], in_=b2.ap().rearrange("(c p) -> p c", p=128))
            w1_g = []
            w2_g = []
            pre_x = {}
            for g in range(W1G):
                t = wts.tile([128, HC, FPER * 128], bf16, tag=f"w1g{g}")
                nc.sync.dma_start(out=t[:], in_=w1.ap()[:, g])
                w1_g.append(t)
                if g == 0:
                    for i, (t0, bs) in enumerate(blocks):
                        if i >= 2:
                            pre_x[i] = emit_x(i, t0, bs, nc.sync)
            for g in range(W2G):
                t = wts.tile([128, CPER, HIDDEN], bf16, tag=f"w2g{g}")
                nc.sync.dma_start(out=t[:], in_=w2.ap()[:, g])
                w2_g.append(t)

            def w1_lhsT(hc, fc):
                return w1_g[fc // FPER][
                    :, hc, (fc % FPER) * 128:(fc % FPER + 1) * 128]

            def w2_lhsT(fc, oc):
                return w2_g[fc // CPER][
                    :, fc % CPER, oc * 128:(oc + 1) * 128]

            for rep in range(repeat):
                # rep 0: x blocks 0-1 + scl on the scalar ring (blocks 2+
                # were pre-issued on sync above). Later reps stream all x
                # on the then-idle sync ring so x isn't queued behind the
                # previous rep's output DMAs.
                xring = nc.scalar if rep == 0 else nc.sync
                x_sb = []
                for i, (t0, bs) in enumerate(blocks):
                    if rep == 0 and i in pre_x:
                        x_sb.append(pre_x[i])
                    else:
                        x_sb.append(emit_x(i, t0, bs, xring))
                s_sb = xin.tile([128, C], f32, tag="s", name="s")
                xring.dma_start(
                    out=s_sb[:], in_=scl.ap()[:].partition_broadcast(128))

                h_sb = hmid.tile([128, FC, C], bf16, tag="h", name="h")

                # Stage B: weight-stationary over (fc, hc); all blocks
                # stream through each loaded weight tile.
                for fc in range(FC):
                    phs = [ps.tile([128, bs], f32, tag="ps",
                                   name=f"ph{fc}_{i}", padded_shape=[128, blk])
                           for i, (_, bs) in enumerate(blocks)]
                    for hc in range(HC):
                        for i, (t0, bs) in enumerate(blocks):
                            nc.tensor.matmul(
                                phs[i][:],
                                w1_lhsT(hc, fc),
                                x_sb[i][:, hc, :],
                                start=(hc == 0), stop=(hc == HC - 1),
                            )
                    for i, (t0, bs) in enumerate(blocks):
                        nc.scalar.activation(
                            out=h_sb[:, fc, t0:t0 + bs], in_=phs[i][:],
                            func=Gelu, bias=b1_sb[:, fc:fc + 1], scale=1.0)

                # Stage C: weight-stationary over (oc, fc).
                for oc in range(HC):
                    pys = [ps.tile([128, bs], f32, tag="ps",
                                   name=f"py{oc}_{i}", padded_shape=[128, blk])
                           for i, (_, bs) in enumerate(blocks)]
                    for fc in range(FC):
                        for i, (t0, bs) in enumerate(blocks):
                            nc.tensor.matmul(
                                pys[i][:],
                                w2_lhsT(fc, oc),
                                h_sb[:, fc, t0:t0 + bs],
                                start=(fc == 0), stop=(fc == FC - 1),
                            )
                    for i, (t0, bs) in enumerate(blocks):
                        o1 = outs.tile([128, bs], f32, tag="o1")
                        nc.scalar.activation(
                            out=o1[:], in_=pys[i][:], func=Ident,
                            bias=b2_sb[:, oc:oc + 1], scale=1.0)
                        nc.vector.tensor_mul(
                            o1[:], o1[:], s_sb[:, t0:t0 + bs])
                        nc.scalar.dma_start(
                            out=yT.ap().rearrange(
                                "(c p) t -> p c t", p=128)[
                                :, oc, t0:t0 + bs],
                            in_=o1[:])

    nc.compile()
    return nc


def _route_host(x, Wr, br):
    """Replicate the reference router bit-exactly (jax on CPU), with a
    numpy fallback (same math, same tie semantics) if jax-cpu is absent."""
    try:
        import jax
        import jax.numpy as jnp

        cpu = jax.devices("cpu")[0]
        xj = jax.device_put(x, cpu)
        Wrj = jax.device_put(Wr, cpu)
        brj = jax.device_put(br, cpu)
        with jax.default_device(cpu):
            logits = jnp.einsum("bsh,he->bse", xj, Wrj) + brj
            routing = jax.nn.softmax(logits, axis=-1)
            topw, topi = jax.lax.top_k(routing, TOP_K)
            topw = jax.nn.softmax(topw, axis=-1)
        return np.asarray(topw), np.asarray(topi)
    except Exception:
        lg = x.reshape(-1, x.shape[-1]).astype(np.float32) @ Wr + br
        m = lg.max(axis=-1, keepdims=True)
        p = np.exp(lg - m)
        p /= p.sum(axis=-1, keepdims=True)
        topi = np.argsort(-p, axis=-1, kind="stable")[:, :TOP_K]
        topv = np.take_along_axis(p, topi, axis=-1)
        e = np.exp(topv - topv.max(axis=-1, keepdims=True))
        topw = (e / e.sum(axis=-1, keepdims=True)).astype(np.float32)
        B, S = x.shape[0], x.shape[1]
        return (topw.reshape(B, S, TOP_K),
                topi.astype(np.int32).reshape(B, S, TOP_K))


def prepare(x, Wr, br, W1, b1, W2, b2, repeat=1):
    """Host-side prep: route, gather, pack. Returns (nc, in_maps, meta)."""
    x = np.ascontiguousarray(np.asarray(x, dtype=np.float32))
    Wr = np.asarray(Wr, dtype=np.float32)
    br = np.asarray(br, dtype=np.float32)
    W1 = np.ascontiguousarray(np.asarray(W1, dtype=np.float32))
    b1 = np.ascontiguousarray(np.asarray(b1, dtype=np.float32))
    W2 = np.ascontiguousarray(np.asarray(W2, dtype=np.float32))
    b2 = np.ascontiguousarray(np.asarray(b2, dtype=np.float32))

    B, S, H = x.shape
    ntok = B * S
    xf = x.reshape(ntok, H)

    topw, topi = _route_host(x, Wr, br)
    topw = topw.reshape(ntok, TOP_K)
    topi = topi.reshape(ntok, TOP_K)

    idx = []
    wgt = []
    for e in range(NUM_EXPERTS):
        mask = (topi == e)
        tok = np.nonzero(mask.any(axis=1))[0]
        w = (topw * mask).sum(axis=1)[tok].astype(np.float32)
        idx.append(tok)
        wgt.append(w)
    counts = np.array([len(t) for t in idx])

    blk = int(os.environ.get("MOE_BLK", "512"))
    C = max(_round_up(int(counts.max()), 2), 512)
    force_c = int(os.environ.get("MOE_FORCE_C", "0"))
    if force_c:
        # perf probe only: clip token counts to the forced C (output will
        # be wrong when force_c < max count; pair with MOE_SKIP_CHECK)
        C = force_c
        counts = np.minimum(counts, C)
        idx = [t[:C] for t in idx]
        wgt = [w[:C] for w in wgt]

    key = (C, blk, repeat)
    if key not in _PROGRAM_CACHE:
        _PROGRAM_CACHE[key] = _build_program(C, blk, repeat=repeat)
    nc = _PROGRAM_CACHE[key]

    import ml_dtypes

    bf16 = ml_dtypes.bfloat16
    in_maps = []
    for e in range(NUM_EXPERTS):
        xTe = np.zeros((H, C), dtype=bf16)
        xTe[:, :counts[e]] = xf[idx[e]].T.astype(bf16)
        scle = np.zeros((C,), dtype=np.float32)
        scle[:counts[e]] = wgt[e]
        W1G, W2G = 8, 8
        HC, FC = H // 128, 2 * H // 128
        FPER, CPER = FC // W1G, FC // W2G
        w1p = np.ascontiguousarray(
            W1[e].astype(bf16).reshape(
                HC, 128, W1G, FPER * 128).transpose(1, 2, 0, 3))
        w2p = np.ascontiguousarray(
            W2[e].astype(bf16).reshape(
                W2G, CPER, 128, H).transpose(2, 0, 1, 3))
        in_maps.append({
            "xT": xTe,
            "w1p": w1p,
            "b1": np.ascontiguousarray(b1[e]),
            "w2p": w2p,
            "b2": np.ascontiguousarray(b2[e]),
            "scl": scle,
        })

    meta = (idx, counts, B, S, H, ntok)
    return nc, in_maps, meta


def combine(results, meta):
    """Host-side unshard: scatter-add per-expert outputs."""
    idx, counts, B, S, H, ntok = meta
    out = np.zeros((ntok, H), dtype=np.float32)
    for e in range(NUM_EXPERTS):
        ye = results[e]["yT"][:, :counts[e]].T  # [cnt, H]
        out[idx[e]] += ye
    return out.reshape(B, S, H)


def kernel(x, Wr, br, W1, b1, W2, b2):
    global LAST_EXEC_NS, LAST_RESULTS
    from concourse.bass_utils import run_bass_kernel_spmd

    nc, in_maps, meta = prepare(x, Wr, br, W1, b1, W2, b2)
    res = run_bass_kernel_spmd(
        nc, in_maps, core_ids=list(range(NCORES)), trace=False)
    LAST_EXEC_NS = res.exec_time_ns
    LAST_RESULTS = res
    return combine(res.results, meta)


# revision 16
# speedup vs baseline: 1.0271x; 1.0271x over previous
"""MoE layer (8 experts, top-2) for 8 Trainium2 NeuronCores — v3.

v2 (bf16, weight-stationary) + two-segment load balancing: each core runs
two expert-pure token segments of static sizes (s0, s1) with separately
streamed weights. The heaviest expert is split across two cores' seg-0
slots and the lightest across two seg-1 slots, so the per-core column
count drops from max_count (2152) to ~s0+s1 (=2067 for the seed-0 data),
recovering most of the expert-imbalance penalty that a one-expert-per-core
SPMD layout must pay.

Per segment: B: h = gelu(x @ W1 + b1); C: yT = (h @ W2 + b2) * scl.
Weights stream just-in-time on the sync ring (w1A, w2A, w1B, w2B in
consumption order, double-buffered via bufs=2 tag rings); x/scl ride the
scalar ring.
"""

import os

import numpy as np

HIDDEN = 1024
FF = 2 * HIDDEN
NUM_EXPERTS = 8
TOP_K = 2
NCORES = 8

LAST_EXEC_NS = None
LAST_RESULTS = None

_PROGRAM_CACHE = {}


def _round_up(v, m):
    return (v + m - 1) // m * m


def _blocks_of(start, size, blk):
    out = []
    t0 = start
    end = start + size
    while t0 < end:
        b = min(blk, end - t0)
        out.append((t0, b))
        t0 += b
    return out


def _build_program(s0, s1, blk, repeat=1):
    """Two-segment expert MLP over s0+s1 tokens (SPMD on 8 cores).

    Layouts (per core):
      xT  [HIDDEN, s0+s1] bf16 - seg0|seg1 tokens, transposed
      w1pA/w1pB [128, 8, HC, 2*128] bf16, b1A/b1B [FF] f32
      w2pA/w2pB [128, 8, 2, HIDDEN] bf16, b2A/b2B [HIDDEN] f32
      scl [s0+s1] f32
      yT  [HIDDEN, s0+s1] f32
    """
    import concourse.bass as bass  # noqa: F401
    import concourse.mybir as mybir
    import concourse.tile as tile
    from concourse import bacc

    HC = HIDDEN // 128
    FC = FF // 128
    f32 = mybir.dt.float32
    bf16 = mybir.dt.bfloat16

    nc = bacc.Bacc("TRN2", target_bir_lowering=False, debug=False,
                   num_devices=NCORES)
    W1G, W2G = 8, 8
    FPER = FC // W1G
    CPER = FC // W2G
    C = s0 + s1
    xT = nc.dram_tensor("xT", [HIDDEN, C], bf16, kind="ExternalInput")
    w1d = [nc.dram_tensor(f"w1p{s}", [128, W1G, HC, FPER * 128], bf16,
                          kind="ExternalInput") for s in "AB"]
    b1d = [nc.dram_tensor(f"b1{s}", [FF], f32, kind="ExternalInput")
           for s in "AB"]
    w2d = [nc.dram_tensor(f"w2p{s}", [128, W2G, CPER, HIDDEN], bf16,
                          kind="ExternalInput") for s in "AB"]
    b2d = [nc.dram_tensor(f"b2{s}", [HIDDEN], f32, kind="ExternalInput")
           for s in "AB"]
    scl = nc.dram_tensor("scl", [C], bf16, kind="ExternalInput")
    yT = nc.dram_tensor("yT", [HIDDEN, C], f32, kind="ExternalOutput")

    seg_blocks = [_blocks_of(0, s0, blk), _blocks_of(s0, s1, blk)]

    Gelu = mybir.ActivationFunctionType.Gelu
    Ident = mybir.ActivationFunctionType.Identity

    with tile.TileContext(nc) as tc:
        with (
            tc.tile_pool(name="wts", bufs=1) as wts,
            tc.tile_pool(name="xin", bufs=1) as xin,
            tc.tile_pool(name="hmid", bufs=1) as hmid,
            tc.tile_pool(name="outs", bufs=4) as outs,
            tc.tile_pool(name="ps", bufs=8, space="PSUM") as ps,
        ):
            # PE clock warm-up (HAM gate 1.2->2.4 GHz needs ~3.4us busy).
            warm = wts.tile([128, 64], bf16, tag="warm")
            nc.vector.memset(warm[:], 0.0)
            for i in range(48):
                pw = ps.tile([128, 64], f32, tag="ps", name=f"warm{i}",
                             padded_shape=[128, blk])
                nc.tensor.matmul(pw[:64, :], warm[:], warm[:],
                                 start=True, stop=True)

            def emit_x(i, t0, bs, ring):
                xc = xin.tile([128, HC, bs], bf16, tag=f"xb{i}",
                              name=f"xb{i}")
                for half in range(2):
                    ring.dma_start(
                        out=xc[:, half * 4:(half + 1) * 4, :],
                        in_=xT.ap().rearrange(
                            "(c p) t -> p c t", p=128)[
                            :, half * 4:(half + 1) * 4, t0:t0 + bs])
                return xc

            # Biases (tiny) first on sync ring.
            b1_sb = []
            b2_sb = []
            for s in range(2):
                t1 = wts.tile([128, FC], f32, tag=f"b1{s}", name=f"b1{s}")
                nc.sync.dma_start(
                    out=t1[:],
                    in_=b1d[s].ap().rearrange("(c p) -> p c", p=128))
                b1_sb.append(t1)
                t2 = wts.tile([128, HC], f32, tag=f"b2{s}", name=f"b2{s}")
                nc.sync.dma_start(
                    out=t2[:],
                    in_=b2d[s].ap().rearrange("(c p) -> p c", p=128))
                b2_sb.append(t2)

            # Weight stream, consumption order: w1A, (seg0 x blocks 2+ on
            # sync between group 0 and 1), w2A, w1B, w2B. bufs=2 tag rings
            # double-buffer A/B so segment B's stream overlaps segment A's
            # compute.
            def emit_w1(dram, tag_suffix, first_extra=None):
                g_tiles = []
                for g in range(W1G):
                    t = wts.tile([128, HC, FPER * 128], bf16,
                                 tag=f"w1g{g}", name=f"w1g{g}{tag_suffix}",
                                 bufs=2)
                    nc.sync.dma_start(out=t[:], in_=dram.ap()[:, g])
                    g_tiles.append(t)
                    if g == 0 and first_extra is not None:
                        first_extra()
                return g_tiles

            def emit_w2(dram, tag_suffix):
                g_tiles = []
                for g in range(W2G):
                    t = wts.tile([128, CPER, HIDDEN], bf16,
                                 tag=f"w2g{g}", name=f"w2g{g}{tag_suffix}",
                                 bufs=2)
                    nc.sync.dma_start(out=t[:], in_=dram.ap()[:, g])
                    g_tiles.append(t)
                return g_tiles

            for rep in range(repeat):
                xring = nc.scalar if rep == 0 else nc.sync
                x_tiles = {}

                def pre_x_seg0():
                    # seg0 x blocks 2+ ride sync between w1A g0 and g1
                    for i, (t0, bs) in enumerate(seg_blocks[0]):
                        if i >= 2:
                            x_tiles[(0, i)] = emit_x(
                                100 * 0 + i, t0, bs, nc.sync)

                if rep == 0:
                    w1A = emit_w1(w1d[0], f"A{rep}", first_extra=pre_x_seg0)
                else:
                    w1A = emit_w1(w1d[0], f"A{rep}")
                for i, (t0, bs) in enumerate(seg_blocks[0]):
                    if (0, i) not in x_tiles:
                        x_tiles[(0, i)] = emit_x(100 * 0 + i, t0, bs, xring)
                for i, (t0, bs) in enumerate(seg_blocks[1]):
                    x_tiles[(1, i)] = emit_x(100 * 1 + i, t0, bs, xring)
                s_sb = xin.tile([128, C], bf16, tag="s", name="s")
                xring.dma_start(
                    out=s_sb[:], in_=scl.ap()[:].partition_broadcast(128))

                w2A = emit_w2(w2d[0], f"A{rep}")
                w1B = emit_w1(w1d[1], f"B{rep}")
                w2B = emit_w2(w2d[1], f"B{rep}")
                seg_w = [(w1A, w2A), (w1B, w2B)]

                for seg in range(2):
                    blocks = seg_blocks[seg]
                    w1_g, w2_g = seg_w[seg]

                    def w1_lhsT(hc, fc):
                        return w1_g[fc // FPER][
                            :, hc, (fc % FPER) * 128:(fc % FPER + 1) * 128]

                    def w2_lhsT(fc, oc):
                        return w2_g[fc // CPER][
                            :, fc % CPER, oc * 128:(oc + 1) * 128]

                    sz = s0 if seg == 0 else s1
                    base = 0 if seg == 0 else s0
                    h_sb = hmid.tile([128, FC, sz], bf16, tag="h",
                                     name=f"h{seg}",
                                     padded_shape=[128, FC, max(s0, s1)])

                    for fc in range(FC):
                        phs = [ps.tile([128, bs], f32, tag="ps",
                                       name=f"ph{seg}_{fc}_{i}",
                                       padded_shape=[128, blk])
                               for i, (_, bs) in enumerate(blocks)]
                        for hc in range(HC):
                            for i, (t0, bs) in enumerate(blocks):
                                nc.tensor.matmul(
                                    phs[i][:],
                                    w1_lhsT(hc, fc),
                                    x_tiles[(seg, i)][:, hc, :bs],
                                    start=(hc == 0), stop=(hc == HC - 1),
                                )
                        for i, (t0, bs) in enumerate(blocks):
                            nc.scalar.activation(
                                out=h_sb[:, fc, t0 - base:t0 - base + bs],
                                in_=phs[i][:],
                                func=Gelu, bias=b1_sb[seg][:, fc:fc + 1],
                                scale=1.0)

                    for oc in range(HC):
                        pys = [ps.tile([128, bs], f32, tag="ps",
                                       name=f"py{seg}_{oc}_{i}",
                                       padded_shape=[128, blk])
                               for i, (_, bs) in enumerate(blocks)]
                        for fc in range(FC):
                            for i, (t0, bs) in enumerate(blocks):
                                nc.tensor.matmul(
                                    pys[i][:],
                                    w2_lhsT(fc, oc),
                                    h_sb[:, fc, t0 - base:t0 - base + bs],
                                    start=(fc == 0), stop=(fc == FC - 1),
                                )
                        for i, (t0, bs) in enumerate(blocks):
                            o1 = outs.tile([128, bs], f32, tag="o1",
                                           padded_shape=[128, blk])
                            nc.scalar.activation(
                                out=o1[:], in_=pys[i][:], func=Ident,
                                bias=b2_sb[seg][:, oc:oc + 1], scale=1.0)
                            nc.vector.tensor_mul(
                                o1[:], o1[:], s_sb[:, t0:t0 + bs])
                            nc.scalar.dma_start(
                                out=yT.ap().rearrange(
                                    "(c p) t -> p c t", p=128)[
                                    :, oc, t0:t0 + bs],
                                in_=o1[:])

    nc.compile()
    return nc


def _route_host(x, Wr, br):
    """Replicate the reference router bit-exactly (jax on CPU), with a
    numpy fallback (same math, same tie semantics) if jax-cpu is absent."""
    try:
        import jax
        import jax.numpy as jnp

        cpu = jax.devices("cpu")[0]
        xj = jax.device_put(x, cpu)
        Wrj = jax.device_put(Wr, cpu)
        brj = jax.device_put(br, cpu)
        with jax.default_device(cpu):
            logits = jnp.einsum("bsh,he->bse", xj, Wrj) + brj
            routing = jax.nn.softmax(logits, axis=-1)
            topw, topi = jax.lax.top_k(routing, TOP_K)
            topw = jax.nn.softmax(topw, axis=-1)
        return np.asarray(topw), np.asarray(topi)
    except Exception:
        lg = x.reshape(-1, x.shape[-1]).astype(np.float32) @ Wr + br
        m = lg.max(axis=-1, keepdims=True)
        p = np.exp(lg - m)
        p /= p.sum(axis=-1, keepdims=True)
        topi = np.argsort(-p, axis=-1, kind="stable")[:, :TOP_K]
        topv = np.take_along_axis(p, topi, axis=-1)
        e = np.exp(topv - topv.max(axis=-1, keepdims=True))
        topw = (e / e.sum(axis=-1, keepdims=True)).astype(np.float32)
        B, S = x.shape[0], x.shape[1]
        return (topw.reshape(B, S, TOP_K),
                topi.astype(np.int32).reshape(B, S, TOP_K))


def _plan_slots(counts):
    """Choose slot sizes (s0, s1) and per-core (expert, range) pairs.

    The heaviest expert spans two seg-0 slots (cores 0,1), the lightest
    spans two seg-1 slots (cores 0,1); the remaining six experts each
    occupy one core's (seg0 + seg1). Minimizes s0+s1 = per-core columns.
    """
    order = list(np.argsort(-np.asarray(counts), kind="stable"))
    emax, emin = order[0], order[-1]
    mids = order[1:-1]
    cmax, cmin = counts[emax], counts[emin]
    s0 = _round_up((cmax + 1) // 2, 2)
    need_mid = max(counts[m] for m in mids) if mids else 0
    s1 = max((cmin + 1) // 2, need_mid - s0, 1)
    s1 = _round_up(s1, 2)

    # per-core: ((expA, startA, lenA), (expB, startB, lenB))
    plans = []
    ha = (cmax + 1) // 2  # emax first-half size
    hb = (cmin + 1) // 2  # emin first-half size
    plans.append(((emax, 0, ha), (emin, 0, hb)))
    plans.append(((emax, ha, cmax - ha), (emin, hb, cmin - hb)))
    for m in mids:
        la = min(s0, counts[m])
        plans.append(((m, 0, la), (m, la, counts[m] - la)))
    return int(s0), int(s1), plans


def prepare(x, Wr, br, W1, b1, W2, b2, repeat=1):
    """Host-side prep: route, plan slots, gather, pack."""
    x = np.ascontiguousarray(np.asarray(x, dtype=np.float32))
    Wr = np.asarray(Wr, dtype=np.float32)
    br = np.asarray(br, dtype=np.float32)
    W1 = np.ascontiguousarray(np.asarray(W1, dtype=np.float32))
    b1 = np.ascontiguousarray(np.asarray(b1, dtype=np.float32))
    W2 = np.ascontiguousarray(np.asarray(W2, dtype=np.float32))
    b2 = np.ascontiguousarray(np.asarray(b2, dtype=np.float32))

    B, S, H = x.shape
    ntok = B * S
    xf = x.reshape(ntok, H)

    topw, topi = _route_host(x, Wr, br)
    topw = topw.reshape(ntok, TOP_K)
    topi = topi.reshape(ntok, TOP_K)

    idx = []
    wgt = []
    for e in range(NUM_EXPERTS):
        mask = (topi == e)
        tok = np.nonzero(mask.any(axis=1))[0]
        w = (topw * mask).sum(axis=1)[tok].astype(np.float32)
        idx.append(tok)
        wgt.append(w)
    counts = [len(t) for t in idx]

    blk = int(os.environ.get("MOE_BLK", "512"))
    s0, s1, plans = _plan_slots(counts)

    key = (s0, s1, blk, repeat)
    if key not in _PROGRAM_CACHE:
        _PROGRAM_CACHE[key] = _build_program(s0, s1, blk, repeat=repeat)
    nc = _PROGRAM_CACHE[key]

    import ml_dtypes

    bf16 = ml_dtypes.bfloat16
    W1G, W2G = 8, 8
    HC, FC = H // 128, 2 * H // 128
    FPER, CPER = FC // W1G, FC // W2G

    wpack = {}

    def packed(e):
        if e not in wpack:
            w1p = np.ascontiguousarray(
                W1[e].astype(bf16).reshape(
                    HC, 128, W1G, FPER * 128).transpose(1, 2, 0, 3))
            w2p = np.ascontiguousarray(
                W2[e].astype(bf16).reshape(
                    W2G, CPER, 128, H).transpose(2, 0, 1, 3))
            wpack[e] = (w1p, w2p)
        return wpack[e]

    C = s0 + s1
    in_maps = []
    for core in range(NCORES):
        (eA, sA, lA), (eB, sB, lB) = plans[core]
        xTe = np.zeros((H, C), dtype=bf16)
        scle = np.zeros((C,), dtype=bf16)
        xTe[:, :lA] = xf[idx[eA][sA:sA + lA]].T.astype(bf16)
        scle[:lA] = wgt[eA][sA:sA + lA]
        xTe[:, s0:s0 + lB] = xf[idx[eB][sB:sB + lB]].T.astype(bf16)
        scle[s0:s0 + lB] = wgt[eB][sB:sB + lB]
        w1pA, w2pA = packed(eA)
        w1pB, w2pB = packed(eB)
        in_maps.append({
            "xT": xTe,
            "w1pA": w1pA, "b1A": np.ascontiguousarray(b1[eA]),
            "w2pA": w2pA, "b2A": np.ascontiguousarray(b2[eA]),
            "w1pB": w1pB, "b1B": np.ascontiguousarray(b1[eB]),
            "w2pB": w2pB, "b2B": np.ascontiguousarray(b2[eB]),
            "scl": scle,
        })

    meta = (idx, plans, s0, B, S, H, ntok)
    return nc, in_maps, meta


def combine(results, meta):
    """Host-side unshard: scatter-add per-(core, segment) outputs."""
    idx, plans, s0, B, S, H, ntok = meta
    out = np.zeros((ntok, H), dtype=np.float32)
    for core in range(NCORES):
        (eA, sA, lA), (eB, sB, lB) = plans[core]
        yv = results[core]["yT"]
        if lA:
            out[idx[eA][sA:sA + lA]] += yv[:, :lA].T
        if lB:
            out[idx[eB][sB:sB + lB]] += yv[:, s0:s0 + lB].T
    return out.reshape(B, S, H)


def kernel(x, Wr, br, W1, b1, W2, b2):
    global LAST_EXEC_NS, LAST_RESULTS
    from concourse.bass_utils import run_bass_kernel_spmd

    nc, in_maps, meta = prepare(x, Wr, br, W1, b1, W2, b2)
    res = run_bass_kernel_spmd(
        nc, in_maps, core_ids=list(range(NCORES)), trace=False)
    LAST_EXEC_NS = res.exec_time_ns
    LAST_RESULTS = res
    return combine(res.results, meta)
